# revision 14
# baseline (speedup 1.0000x reference)
"""Trainium2 Bass kernel for BaselineMoE (top-6-of-32 routed experts + 2 shared).

Strategy (8 NeuronCores, expert-parallel per the sharding hint):
  - Host computes the (cheap) router softmax/top-k from the actual inputs,
    gathers each expert's tokens into a padded, transposed buffer, and deals
    the 32 routed experts across 8 cores x 4 slots, balancing per-core load.
  - Each core runs a dense SwiGLU MLP (gate/up/down, sigmoid(gate)*up) for its
    4 routed experts on the pre-gathered tokens, with the per-token top-k gate
    weights applied on-device during PSUM evacuation.
  - Routed experts run in fp8e4 with DoubleRow matmuls (2 contraction rows per
    PE cell) using power-of-2 scales folded into the sigmoid input scale and
    the gate coefficients; PSUM accumulation stays f32.
  - The 2 shared experts are split across core halves (cores 0-3 run shared
    expert 0, cores 4-7 expert 1, each on a 512-token shard) in bf16 — they
    carry most of the output magnitude, so they stay higher precision, and the
    split halves the shared-weight HBM traffic vs full replication.
  - Each tensor is loaded/stored with a single large rearranged-AP DMA
    (~32 DMAs per pass total) to keep the DGE issue path off the critical
    path. Expert outputs come back bf16; the host scatter-adds them into the
    residual stream in f32.

Capacities (per-slot token counts) are computed from the actual routing at
call time, so the emitted program adapts to the input.
"""

from contextlib import ExitStack

import numpy as np
import ml_dtypes

import concourse.bacc as bacc
import concourse.tile as tile
import concourse.mybir as mybir
from concourse.bass_utils import run_bass_kernel_spmd

H = 2048
I = 1024
E = 32
NS = 2
TOP_K = 6
SCALE = 1.0
NCORES = 8
SLOTS = 4          # routed experts per core
TSH = 512          # shared-expert tokens per core (T / 4; 2-way expert split)
KH = H // 128      # 16 k-tiles over H
KI = I // 128      # 8 k-tiles over I
PH = H // 256      # 8 double-row pairs over H
PI = I // 256      # 4 double-row pairs over I
BF16 = mybir.dt.bfloat16
F32 = mybir.dt.float32
FP8 = mybir.dt.float8e4
NP_FP8 = mybir.dt.np(FP8)

# power-of-2 fp8 scales (descales are folded into sigmoid scale / gates).
# fp8e4 here is IEEE e4m3 (max finite 240): z = sigmoid(g) * u carries
# S_WU * S_X = 32x and must stay well under 240 when cast to fp8.
S_X = 8.0          # tokens
S_WG = 8.0         # gate weights
S_WU = 4.0         # up weights
S_WD = 32.0        # down weights
DESCALE_GATE = 1.0 / (S_WG * S_X)                    # on sigmoid input
DESCALE_Y = 1.0 / (S_WU * S_X * S_WD)                # folded into gates

_PROGRAM_CACHE: dict = {}


def _to_bf16(a: np.ndarray) -> np.ndarray:
    """f32 -> bf16 with round-to-nearest-even (fast uint trick)."""
    a = np.ascontiguousarray(a, dtype=np.float32)
    u = a.view(np.uint32)
    r = (u + np.uint32(0x7FFF) + ((u >> np.uint32(16)) & np.uint32(1))) >> np.uint32(16)
    return r.astype(np.uint16).view(ml_dtypes.bfloat16)


def _fp8_pairs(a: np.ndarray, scale: float) -> np.ndarray:
    """[K, N] f32 -> [K/256, 128, 2, N] fp8e4, DoubleRow-interleaved."""
    K, N = a.shape
    q = (np.asarray(a, np.float32) * scale).reshape(K // 256, 2, 128, N)
    return np.ascontiguousarray(q.transpose(0, 2, 1, 3)).astype(NP_FP8)


def _route(flat: np.ndarray, Wr: np.ndarray):
    """Host router: softmax over experts, exact top-k gate mask."""
    logits = flat.astype(np.float32) @ Wr.astype(np.float32)
    m = logits.max(axis=-1, keepdims=True)
    p = np.exp(logits - m)
    p /= p.sum(axis=-1, keepdims=True)
    T = p.shape[0]
    idx = np.argpartition(-p, TOP_K - 1, axis=-1)[:, :TOP_K]
    gates = np.zeros((T, E), np.float32)
    rows = np.arange(T)[:, None]
    gates[rows, idx] = p[rows, idx] * SCALE
    return gates


def _assign_experts(tok_idx):
    """Deal experts into (core, slot) balancing per-core token totals.

    Experts with more than 512 tokens (the PSUM-bank N limit) are split into
    pseudo-experts with disjoint token chunks, so slot capacity never exceeds
    512. Slot s holds the pseudo-experts ranked [8s, 8s+8) by token count;
    within a slot the largest goes to the least-loaded core. Returns
    (assign, caps, chunks) where chunks[j] = (expert, token_index_array) and
    assign[core][slot] indexes into chunks (-1 = empty).
    """
    chunks = []
    for e, ix in enumerate(tok_idx):
        for off in range(0, max(len(ix), 1), 512):
            chunks.append((e, ix[off:off + 512]))
    while len(chunks) % NCORES:
        chunks.append((0, np.zeros(0, np.int32)))
    counts = np.array([len(ix) for _, ix in chunks], np.int64)
    n_slots = len(chunks) // NCORES
    order = np.argsort(-counts, kind="stable")
    assign = [[-1] * n_slots for _ in range(NCORES)]
    load = np.zeros(NCORES, np.int64)
    caps = []
    for s in range(n_slots):
        group = list(order[s * NCORES:(s + 1) * NCORES])
        caps.append(int(counts[group].max()) if group else 0)
        for j in group:  # descending count; give to least-loaded core
            c = int(np.argmin(load))
            assign[c][s] = int(j)
            load[c] += counts[j]
    caps = [min(512, max(64, -(-c // 16) * 16)) for c in caps]
    return assign, caps, chunks


def build_program(caps, loop_reps=None):
    """Build the per-core Bass program for the given slot capacities.

    loop_reps: if set, wrap the whole body in a device-side For_i loop —
    used by the test harness to amplify exec time above dispatch overhead.
    """
    caps = tuple(int(c) for c in caps)
    key = (caps, loop_reps)
    if key in _PROGRAM_CACHE:
        return _PROGRAM_CACHE[key]

    nc = bacc.Bacc("TRN2", target_bir_lowering=False, debug=False)

    xg_d, wg_d, wu_d, wd_d, g_d, y_d = [], [], [], [], [], []
    for s in range(len(caps)):
        C = caps[s]
        xg_d.append(nc.dram_tensor(f"xg{s}", [PH, 128, 2, C], FP8, kind="ExternalInput"))
        wg_d.append(nc.dram_tensor(f"wg{s}", [PH, 128, 2, I], FP8, kind="ExternalInput"))
        wu_d.append(nc.dram_tensor(f"wu{s}", [PH, 128, 2, I], FP8, kind="ExternalInput"))
        wd_d.append(nc.dram_tensor(f"wd{s}", [PI, 128, 2, H], FP8, kind="ExternalInput"))
        g_d.append(nc.dram_tensor(f"g{s}", [1, C], F32, kind="ExternalInput"))
        y_d.append(nc.dram_tensor(f"y{s}", [KH, 128, C], BF16, kind="ExternalOutput"))
    xs_d = nc.dram_tensor("xs", [KH, 128, TSH], BF16, kind="ExternalInput")
    wgs_d = nc.dram_tensor("wgs", [KH, 128, I], BF16, kind="ExternalInput")
    wus_d = nc.dram_tensor("wus", [KH, 128, I], BF16, kind="ExternalInput")
    wds_d = nc.dram_tensor("wds", [KI, 128, H], BF16, kind="ExternalInput")
    ys_d = nc.dram_tensor("ys", [KH, 128, TSH], F32, kind="ExternalOutput")

    DR = mybir.MatmulPerfMode.DoubleRow

    with tile.TileContext(nc) as tc:
        with (
            tc.tile_pool(name="w", bufs=2) as wpool,
            tc.tile_pool(name="xg", bufs=2) as xpool,
            tc.tile_pool(name="gb", bufs=2) as gbpool,
            tc.tile_pool(name="sg", bufs=2) as sgpool,
            tc.tile_pool(name="z", bufs=2) as zpool,
            tc.tile_pool(name="o", bufs=2) as opool,
            tc.tile_pool(name="os", bufs=2) as ospool,
            tc.tile_pool(name="pg", bufs=2, space="PSUM") as pgpool,
            tc.tile_pool(name="pu", bufs=2, space="PSUM") as pupool,
            tc.tile_pool(name="py", bufs=4, space="PSUM") as pypool,
            ExitStack() as stack,
        ):
            if loop_reps is not None:
                stack.enter_context(tc.For_i(0, loop_reps, 1))

            # ---- routed experts: fp8 DoubleRow ----
            for s in range(len(caps)):
                C = caps[s]
                xg_t = xpool.tile([128, PH, 2, C], FP8, tag="xg")
                nc.sync.dma_start(xg_t[:], xg_d[s][:].rearrange("p q r c -> q p r c"))
                gb = gbpool.tile([128, C], F32, tag="gb")
                nc.sync.dma_start(gb[:], g_d[s][:].partition_broadcast(128))

                wg_t = wpool.tile([128, PH, 2, I], FP8, tag="w")
                nc.sync.dma_start(wg_t[:], wg_d[s][:].rearrange("p q r i -> q p r i"))
                sg = sgpool.tile([128, KI, C], BF16, tag="sg")
                for m in range(KI):
                    pg = pgpool.tile([128, C], F32, tag="pg")
                    for p in range(PH):
                        nc.tensor.matmul(pg[:], wg_t[:, p, :, m * 128:(m + 1) * 128],
                                         xg_t[:, p], start=(p == 0), stop=(p == PH - 1),
                                         perf_mode=DR)
                    nc.scalar.activation(sg[:, m, :], pg[:],
                                         mybir.ActivationFunctionType.Sigmoid,
                                         scale=DESCALE_GATE)

                wu_t = wpool.tile([128, PH, 2, I], FP8, tag="w")
                nc.sync.dma_start(wu_t[:], wu_d[s][:].rearrange("p q r i -> q p r i"))
                z = zpool.tile([128, KI, C], FP8, tag="z")
                for m in range(KI):
                    pu = pupool.tile([128, C], F32, tag="pu")
                    for p in range(PH):
                        nc.tensor.matmul(pu[:], wu_t[:, p, :, m * 128:(m + 1) * 128],
                                         xg_t[:, p], start=(p == 0), stop=(p == PH - 1),
                                         perf_mode=DR)
                    nc.vector.tensor_mul(z[:, m, :], sg[:, m, :], pu[:])

                wd_t = wpool.tile([128, PI, 2, H], FP8, tag="w")
                nc.sync.dma_start(wd_t[:], wd_d[s][:].rearrange("p q r h -> q p r h"))
                ot = opool.tile([128, KH, C], BF16, tag="o")
                for h in range(KH):
                    py = pypool.tile([128, C], F32, tag="py")
                    for p in range(PI):
                        nc.tensor.matmul(py[:], wd_t[:, p, :, h * 128:(h + 1) * 128],
                                         z[:, 2 * p:2 * p + 2, :], start=(p == 0),
                                         stop=(p == PI - 1), perf_mode=DR)
                    nc.vector.tensor_mul(ot[:, h, :], py[:], gb[:])
                nc.sync.dma_start(y_d[s][:].rearrange("h q c -> q h c"), ot[:])

            # ---- shared expert (one per core half): bf16 ----
            xs_t = xpool.tile([128, KH, TSH], BF16, tag="xg")
            nc.sync.dma_start(xs_t[:], xs_d[:].rearrange("k q t -> q k t"))
            wg_t = wpool.tile([128, KH, I], BF16, tag="w")
            nc.sync.dma_start(wg_t[:], wgs_d[:].rearrange("k q i -> q k i"))
            sg = sgpool.tile([128, KI, TSH], BF16, tag="sg")
            for m in range(KI):
                pg = pgpool.tile([128, TSH], F32, tag="pg")
                for k in range(KH):
                    nc.tensor.matmul(pg[:], wg_t[:, k, m * 128:(m + 1) * 128],
                                     xs_t[:, k, :], start=(k == 0), stop=(k == KH - 1))
                nc.scalar.activation(sg[:, m, :], pg[:],
                                     mybir.ActivationFunctionType.Sigmoid)
            wu_t = wpool.tile([128, KH, I], BF16, tag="w")
            nc.sync.dma_start(wu_t[:], wus_d[:].rearrange("k q i -> q k i"))
            zb = zpool.tile([128, KI, TSH], BF16, tag="z")
            for m in range(KI):
                pu = pupool.tile([128, TSH], F32, tag="pu")
                for k in range(KH):
                    nc.tensor.matmul(pu[:], wu_t[:, k, m * 128:(m + 1) * 128],
                                     xs_t[:, k, :], start=(k == 0), stop=(k == KH - 1))
                nc.vector.tensor_mul(zb[:, m, :], sg[:, m, :], pu[:])
            wd_t = wpool.tile([128, KI, H], BF16, tag="w")
            nc.sync.dma_start(wd_t[:], wds_d[:].rearrange("j q h -> q j h"))
            for hg in range(KH // 4):
                os_t = ospool.tile([128, 4, TSH], F32, tag="os")
                for hh in range(4):
                    h = hg * 4 + hh
                    py = pypool.tile([128, TSH], F32, tag="py")
                    for j in range(KI):
                        nc.tensor.matmul(py[:], wd_t[:, j, h * 128:(h + 1) * 128],
                                         zb[:, j, :], start=(j == 0), stop=(j == KI - 1))
                    nc.vector.tensor_copy(os_t[:, hh, :], py[:])
                nc.sync.dma_start(ys_d[hg * 4:(hg + 1) * 4].rearrange("h q t -> q h t"),
                                  os_t[:])

    nc.compile()
    _PROGRAM_CACHE[key] = nc
    return nc


def prepare(x, Wr, Wg_s, Wu_s, Wd_s, Wg, Wu, Wd):
    """Host-side routing, sharding and fp8/bf16 packing. Returns (nc, in_maps, meta)."""
    flat = np.ascontiguousarray(x, np.float32).reshape(-1, H)
    T = flat.shape[0]
    assert T == 4 * TSH

    gates = _route(flat, Wr)
    tok_idx = [np.nonzero(gates[:, e])[0].astype(np.int32) for e in range(E)]
    assign, caps, chunks = _assign_experts(tok_idx)

    nc = build_program(caps)

    xT = np.ascontiguousarray(flat.T)          # [H, T] f32
    wgs_b = [_to_bf16(Wg_s[e]).reshape(KH, 128, I) for e in range(NS)]
    wus_b = [_to_bf16(Wu_s[e]).reshape(KH, 128, I) for e in range(NS)]
    wds_b = [_to_bf16(Wd_s[e]).reshape(KI, 128, H) for e in range(NS)]
    xs_b = [_to_bf16(xT[:, p * TSH:(p + 1) * TSH]).reshape(KH, 128, TSH)
            for p in range(4)]

    in_maps = []
    for c in range(NCORES):
        half, part = divmod(c, 4)
        im = {"wgs": wgs_b[half], "wus": wus_b[half], "wds": wds_b[half],
              "xs": xs_b[part]}
        for s in range(len(caps)):
            e, ix = chunks[assign[c][s]]
            C = caps[s]
            xg = np.zeros((H, C), np.float32)
            xg[:, :len(ix)] = xT[:, ix]
            im[f"xg{s}"] = _fp8_pairs(xg, S_X)
            g = np.zeros((1, C), np.float32)
            g[0, :len(ix)] = gates[ix, e] * DESCALE_Y
            im[f"g{s}"] = g
            im[f"wg{s}"] = _fp8_pairs(np.asarray(Wg[e]), S_WG)
            im[f"wu{s}"] = _fp8_pairs(np.asarray(Wu[e]), S_WU)
            im[f"wd{s}"] = _fp8_pairs(np.asarray(Wd[e]), S_WD)
        in_maps.append(im)

    meta = {"assign": assign, "caps": caps, "chunks": chunks,
            "flat": flat, "shape": x.shape}
    return nc, in_maps, meta


def postprocess(results, meta):
    """Scatter-add per-expert outputs + shared shards + residual."""
    flat = meta["flat"]
    out = flat.copy()
    for c in range(NCORES):
        part = c % 4
        sh = results[c]["ys"].reshape(H, TSH)
        out[part * TSH:(part + 1) * TSH] += sh.T
        for s in range(len(meta["caps"])):
            _, ix = meta["chunks"][meta["assign"][c][s]]
            if len(ix) == 0:
                continue
            Y = results[c][f"y{s}"].reshape(H, meta["caps"][s])
            out[ix] += Y[:, :len(ix)].T.astype(np.float32)
    return out.reshape(meta["shape"]).astype(np.float32, copy=False)


def kernel(x, Wr, Wg_s, Wu_s, Wd_s, Wg, Wu, Wd):
    nc, in_maps, meta = prepare(x, Wr, Wg_s, Wu_s, Wd_s, Wg, Wu, Wd)
    res = run_bass_kernel_spmd(nc, in_maps, list(range(NCORES)))
    return postprocess(res.results, meta)
